# revision 22
# baseline (speedup 1.0000x reference)
"""DistanceWeightedAttention Trainium2 kernel (8 NeuronCores, SPMD).

Strategy (degree-sorted row layout):
  - Rows (query nodes) with deg>0 are sorted by degree and dealt round-robin
    to the 8 cores, so every core sees an identical degree profile and the
    SPMD program (one bass module for all cores) has a common bin template.
  - A bin = 128 rows (partition dim) x D slots (free dim), D = max degree in
    the bin; degree sorting makes padding negligible. Edge (row j, slot s) of
    bin b sits at gather position (slotbase_b + s)*128 + j, so a single
    SWDGE gather of [K|V] fp16 rows (512B descriptors -- the cost-model
    sweet spot) lands kve[j, slot, 0:256] with partition = row.
  - Per bin, on-device:
      prod = K_e * q_row (DVE fp16, row broadcast over slots, 2x mode)
      scores = pairwise-add cascade 32->16->8->4->2->1 (all fp16 TT, 2x)
      scores = scores * rbf + mask  (pads get -80 -> exp == 0 in fp16)
      exps = ACT Exp; den = sum_s exps via PE identity-matmul accumulation
      rec = 1/(den+1e-8); exn = exps*rec; pair-duplicated exps2 so the
      wv multiply keeps a stride-1 last dim (DVE 2x mode)
      wv = V_e * probs; outU^T accumulated with matmul(lhsT=wv, rhs=I)
      out = outU^T.T @ Wo via one matmul per bin (bo added on host)
  - K/V/Q projections are computed on device in fp16 (biases as rank-1
    matmuls); kvtab ([K|V] per dst node) is staged in DRAM fp16.
  - Softmax uses the unstable form exp(s)/(sum exp(s)+1e-8); scores are O(1)
    here so this matches the reference's max(0,segmax) form to ~1e-7.
"""

import sys

import numpy as np

sys.path.insert(0, "/opt/trn_rl_repo")

HIDDEN = 128
HEADS = 4
HD = 32
SCALE = float(np.sqrt(HD))
NCORES = 8
MASK_PAD = -80.0
MAX_GROUP_SLOTS = 24      # slot-chunks per gather group cap (SBUF)
MAX_GROUP_BINS = 4

_PROG_CACHE = {}


def _build_program(nkv_pad, nbins, D_list, groups, total_slots):
    import concourse.bass as bass
    import concourse.bacc as bacc
    import concourse.tile as tile
    from concourse import mybir

    f32 = mybir.dt.float32
    f16 = mybir.dt.float16
    i16 = mybir.dt.int16
    nkv_tiles = nkv_pad // 128
    KSLAB = 24
    maxD = max(D_list)
    max_gslots = max(sum(D_list[b] for b in g) for g in groups)
    slotbase = np.concatenate([[0], np.cumsum(D_list)]).astype(int)

    nc = bacc.Bacc("TRN2", target_bir_lowering=False, debug=False,
                   num_devices=NCORES)

    t_qT = nc.dram_tensor("qT", [128, nbins * 128], f16, kind="ExternalInput")
    t_kT = nc.dram_tensor("kT", [128, nkv_pad], f16, kind="ExternalInput")
    t_vT = nc.dram_tensor("vT", [128, nkv_pad], f16, kind="ExternalInput")
    t_Wq = nc.dram_tensor("Wq", [128, 128], f16, kind="ExternalInput")
    t_Wk = nc.dram_tensor("Wk", [128, 128], f16, kind="ExternalInput")
    t_Wv = nc.dram_tensor("Wv", [128, 128], f16, kind="ExternalInput")
    t_Wo = nc.dram_tensor("Wo", [128, 128], f16, kind="ExternalInput")
    t_bq = nc.dram_tensor("bq", [1, 128], f16, kind="ExternalInput")
    t_bk = nc.dram_tensor("bk", [1, 128], f16, kind="ExternalInput")
    t_bv = nc.dram_tensor("bv", [1, 128], f16, kind="ExternalInput")
    t_ones = nc.dram_tensor("ones1", [1, 128], f16, kind="ExternalInput")
    t_bkv4 = nc.dram_tensor("bkv4", [1, 512], f16, kind="ExternalInput")
    t_bq4 = nc.dram_tensor("bq4", [1, 512], f16, kind="ExternalInput")
    t_ident = nc.dram_tensor("ident", [128, 128], f16, kind="ExternalInput")
    t_rbfm = nc.dram_tensor("rbfm", [128, total_slots * HEADS], f16,
                            kind="ExternalInput")
    t_pcnt = nc.dram_tensor("pcnt", [128, nbins], f32,
                            kind="ExternalInput")
    t_didx = nc.dram_tensor("didx", [128, total_slots * 8], i16,
                            kind="ExternalInput")
    t_out = nc.dram_tensor("out", [128, nbins * 128], f16,
                           kind="ExternalOutput")

    with tile.TileContext(nc) as tc:
        with (
            tc.tile_pool(name="const", bufs=1) as constp,
            tc.tile_pool(name="slab", bufs=2) as slabp,
            tc.tile_pool(name="work", bufs=3) as work,
            tc.tile_pool(name="kve", bufs=4) as kvep,
            tc.tile_pool(name="edge", bufs=4) as edgep,
            tc.tile_pool(name="sm", bufs=8) as smp,
            tc.tile_pool(name="fin", bufs=3) as finp,
            tc.tile_pool(name="ps", bufs=3, space="PSUM") as psp,
            tc.tile_pool(name="dps", bufs=2, space="PSUM") as dpsp,
            tc.tile_pool(name="ops", bufs=3, space="PSUM") as opsp,
            tc.tile_pool(name="dram", bufs=1, space="DRAM") as dramp,
        ):
            Wq = constp.tile([128, 128], f16, tag="Wq")
            Wk = constp.tile([128, 128], f16, tag="Wk")
            Wv = constp.tile([128, 128], f16, tag="Wv")
            Wo = constp.tile([128, 128], f16, tag="Wo")
            bq = constp.tile([1, 128], f16, tag="bq")
            bk = constp.tile([1, 128], f16, tag="bk")
            bv = constp.tile([1, 128], f16, tag="bv")
            ones = constp.tile([1, 128], f16, tag="ones")
            bkv4 = constp.tile([1, 512], f16, tag="bkv4")
            bq4 = constp.tile([1, 512], f16, tag="bq4")
            ident = constp.tile([128, 128], f16, tag="ident")
            rbfm = constp.tile([128, total_slots * HEADS], f16, tag="rbfm")
            pcnt = constp.tile([128, nbins], f32, tag="pcnt")
            didx = constp.tile([128, total_slots * 8], i16, tag="didx")
            qproj = constp.tile([128, nbins * 128], f16, tag="qproj")
            nc.sync.dma_start(Wq[:], t_Wq[:])
            nc.sync.dma_start(Wk[:], t_Wk[:])
            nc.sync.dma_start(Wv[:], t_Wv[:])
            nc.sync.dma_start(Wo[:], t_Wo[:])
            nc.sync.dma_start(bq[:], t_bq[:])
            nc.sync.dma_start(bk[:], t_bk[:])
            nc.sync.dma_start(bv[:], t_bv[:])
            nc.sync.dma_start(ones[:], t_ones[:])
            nc.sync.dma_start(bkv4[:], t_bkv4[:])
            nc.sync.dma_start(bq4[:], t_bq4[:])
            nc.sync.dma_start(ident[:], t_ident[:])
            nc.scalar.dma_start(rbfm[:], t_rbfm[:])
            nc.scalar.dma_start(pcnt[:], t_pcnt[:])
            nc.scalar.dma_start(didx[:], t_didx[:])
            rbf_v = rbfm[:].rearrange("p (s h) -> p s h", h=HEADS)

            kvtab = dramp.tile([nkv_pad + 128, 256], f16, tag="kvtab")
            # dedicated all-zero row block at [nkv_pad] for pad slots
            zrow = work.tile([128, 512], f16, tag="zrow")
            nc.vector.memset(zrow[:], 0.0)
            nc.gpsimd.dma_start(
                kvtab[nkv_pad:nkv_pad + 128, :].rearrange(
                    "(t p) f -> p t f", p=128),
                zrow[:, 0:256].rearrange("p (t f) -> p t f", f=256))

            # ---- K/V projection -> kvtab fp16 ---------------------------
            for s0 in range(0, nkv_tiles, KSLAB):
                nt = min(KSLAB, nkv_tiles - s0)
                ksl = slabp.tile([128, KSLAB * 128], f16, tag="ksl")
                vsl = slabp.tile([128, KSLAB * 128], f16, tag="vsl")
                nc.sync.dma_start(ksl[:, 0:nt * 128],
                                  t_kT[:, s0 * 128:(s0 + nt) * 128])
                nc.scalar.dma_start(vsl[:, 0:nt * 128],
                                    t_vT[:, s0 * 128:(s0 + nt) * 128])
                for g0 in range(0, nt, 4):
                    ng = min(4, nt - g0)
                    kvsb = work.tile([128, 1024], f16, tag="kvsb")
                    for h0 in range(0, ng, 2):
                        kvps = psp.tile([128, 512], f32, tag="mm")
                        nc.tensor.matmul(kvps[:], ones[:], bkv4[:],
                                         start=True, stop=False)
                        for i in range(2):
                            t = g0 + h0 + i
                            if t >= nt:
                                continue
                            lo = i * 256
                            nc.tensor.matmul(kvps[:, lo:lo + 128],
                                             ksl[:, t * 128:(t + 1) * 128],
                                             Wk[:], start=False, stop=True)
                            nc.tensor.matmul(kvps[:, lo + 128:lo + 256],
                                             vsl[:, t * 128:(t + 1) * 128],
                                             Wv[:], start=False, stop=True)
                        nc.scalar.copy(kvsb[:, h0 * 256:h0 * 256 + 512],
                                       kvps[:])
                    nc.gpsimd.dma_start(
                        kvtab[(s0 + g0) * 128:(s0 + g0 + ng) * 128, :]
                        .rearrange("(t p) f -> p t f", p=128),
                        kvsb[:, 0:ng * 256].rearrange(
                            "p (t f) -> p t f", f=256))

            # ---- Q projection (emitted slab-wise, interleaved) ----------
            QSLAB = 16

            def q_slab(q0):
                nq_t = min(QSLAB, nbins - q0)
                qsl = slabp.tile([128, QSLAB * 128], f16, tag="qsl")
                nc.sync.dma_start(qsl[:, 0:nq_t * 128],
                                  t_qT[:, q0 * 128:(q0 + nq_t) * 128])
                for i0 in range(0, nq_t, 4):
                    nb = min(4, nq_t - i0)
                    qps = psp.tile([128, 512], f32, tag="mm")
                    nc.tensor.matmul(qps[:], ones[:], bq4[:],
                                     start=True, stop=False)
                    for i in range(nb):
                        lo = i * 128
                        nc.tensor.matmul(qps[:, lo:lo + 128],
                                         qsl[:, (i0 + i) * 128:(i0 + i + 1) * 128],
                                         Wq[:], start=False, stop=True)
                    nc.scalar.copy(qproj[:, (q0 + i0) * 128:(q0 + i0 + nb) * 128],
                                   qps[:, 0:nb * 128])

            # ---- main edge loop (software-pipelined: stage A of
            # group g+1 is emitted before stage B of group g so in-order
            # engine queues never head-of-line block on exp/den results) ---
            GS = max_gslots
            state = {}

            def stage_a(gi):
                g = groups[gi]
                gs0 = slotbase[g[0]]
                G_ = sum(D_list[b] for b in g)
                kve = kvep.tile([128, GS, 256], f16, tag="kve")
                nc.gpsimd.dma_gather(
                    out_ap=kve[:, 0:G_, :], in_ap=kvtab[:],
                    idxs_ap=didx[:, gs0 * 8:(gs0 + G_) * 8],
                    num_idxs=G_ * 128, num_idxs_reg=G_ * 128,
                    elem_size=256, single_packet=False,
                )
                prod = edgep.tile([128, GS, 128], f16, tag="prod")
                for b in g:
                    D = D_list[b]
                    sb0 = slotbase[b] - gs0
                    qb = qproj[:, b * 128:(b + 1) * 128].unsqueeze(1)
                    nc.vector.tensor_tensor(
                        prod[:, sb0:sb0 + D, :], kve[:, sb0:sb0 + D, 0:128],
                        qb.broadcast_to([128, D, 128]),
                        op=mybir.AluOpType.mult)
                casc = edgep.tile([128, GS * 64], f16, tag="casc")
                cv = casc[:].rearrange("p (s h d) -> p s h d", h=HEADS, d=16)
                pv = prod[:].rearrange("p s (h d) -> p s h d", d=HD)
                nc.vector.tensor_tensor(
                    cv[:, 0:G_, :, 0:16], pv[:, 0:G_, :, 0:16],
                    pv[:, 0:G_, :, 16:32], op=mybir.AluOpType.add)
                nc.vector.tensor_tensor(
                    cv[:, 0:G_, :, 0:8], cv[:, 0:G_, :, 0:8],
                    cv[:, 0:G_, :, 8:16], op=mybir.AluOpType.add)
                nc.vector.tensor_tensor(
                    cv[:, 0:G_, :, 0:4], cv[:, 0:G_, :, 0:4],
                    cv[:, 0:G_, :, 4:8], op=mybir.AluOpType.add)
                nc.vector.tensor_tensor(
                    cv[:, 0:G_, :, 0:2], cv[:, 0:G_, :, 0:2],
                    cv[:, 0:G_, :, 2:4], op=mybir.AluOpType.add)
                sm = smp.tile([128, GS * HEADS], f16, tag="sm")
                smv = sm[:].rearrange("p (s h) -> p s h", h=HEADS)
                nc.vector.tensor_tensor(
                    smv[:, 0:G_, :], cv[:, 0:G_, :, 0],
                    cv[:, 0:G_, :, 1], op=mybir.AluOpType.add)
                nc.vector.tensor_tensor(
                    smv[:, 0:G_, :], smv[:, 0:G_, :],
                    rbf_v[:, gs0:gs0 + G_, :], op=mybir.AluOpType.mult)
                exps = smp.tile([128, GS * HEADS], f16, tag="exps")
                ev = exps[:].rearrange("p (s h) -> p s h", h=HEADS)
                nc.scalar.activation(ev[:, 0:G_, :], smv[:, 0:G_, :],
                                     mybir.ActivationFunctionType.Exp)
                # den accumulation for all bins of the group -> one psum
                dps = dpsp.tile([128, MAX_GROUP_BINS * HEADS], f32, tag="den")
                for j, b in enumerate(g):
                    D = D_list[b]
                    sb0 = slotbase[b] - gs0
                    for s in range(D):
                        nc.tensor.matmul(dps[:, j * HEADS:(j + 1) * HEADS],
                                         ident[:], ev[:, sb0 + s, :],
                                         start=(s == 0), stop=(s == D - 1))
                state[gi] = (kve, exps, dps)

            def stage_b(gi):
                g = groups[gi]
                gs0 = slotbase[g[0]]
                G_ = sum(D_list[b] for b in g)
                ng = len(g)
                kve, exps, dps = state.pop(gi)
                ev = exps[:].rearrange("p (s h) -> p s h", h=HEADS)
                den = smp.tile([128, MAX_GROUP_BINS * HEADS], f32, tag="densb")
                for j, b in enumerate(g):
                    nc.vector.tensor_scalar(
                        den[:, j * HEADS:(j + 1) * HEADS],
                        dps[:, j * HEADS:(j + 1) * HEADS],
                        pcnt[:, b:b + 1], 1e-8,
                        op0=mybir.AluOpType.subtract, op1=mybir.AluOpType.add)
                rec = smp.tile([128, MAX_GROUP_BINS * HEADS], f32, tag="rec")
                nc.vector.reciprocal(rec[:, 0:ng * HEADS],
                                     den[:, 0:ng * HEADS])
                rv = rec[:].rearrange("p (j h) -> p j h", h=HEADS)
                ex2 = smp.tile([128, GS * HEADS * 2], f16, tag="ex2")
                e2v = ex2[:].rearrange("p (s h two) -> p s h two", h=HEADS,
                                       two=2)
                oUTps = opsp.tile([128, MAX_GROUP_BINS * 128], f32, tag="oUT")
                wv = edgep.tile([128, GS, 128], f16, tag="wv")
                wvv = wv[:].rearrange("p s (h d two) -> p s h d two",
                                      h=HEADS, d=16, two=2)
                for j, b in enumerate(g):
                    D = D_list[b]
                    sb0 = slotbase[b] - gs0
                    nc.vector.tensor_tensor(
                        e2v[:, sb0:sb0 + D, :, :],
                        ev[:, sb0:sb0 + D, :].unsqueeze(3).broadcast_to(
                            [128, D, HEADS, 2]),
                        rv[:, j, :].unsqueeze(1).unsqueeze(3).broadcast_to(
                            [128, D, HEADS, 2]),
                        op=mybir.AluOpType.mult)
                    vv = kve[:, sb0:sb0 + D, 128:256].rearrange(
                        "p s (h d two) -> p s h d two", h=HEADS, d=16, two=2)
                    e2b = e2v[:, sb0:sb0 + D, :, :].unsqueeze(3).broadcast_to(
                        [128, D, HEADS, 16, 2])
                    nc.vector.tensor_tensor(wvv[:, sb0:sb0 + D], vv, e2b,
                                            op=mybir.AluOpType.mult)
                    for s in range(D):
                        nc.tensor.matmul(
                            oUTps[:, j * 128:(j + 1) * 128],
                            wv[:, sb0 + s, :], ident[:],
                            start=(s == 0), stop=(s == D - 1))
                oUT = finp.tile([128, MAX_GROUP_BINS * 128], f16, tag="oUTsb")
                nc.scalar.copy(oUT[:, 0:ng * 128], oUTps[:, 0:ng * 128])
                fps = psp.tile([128, 512], f32, tag="mm")
                for j in range(ng):
                    nc.tensor.matmul(fps[:, j * 128:(j + 1) * 128],
                                     oUT[:, j * 128:(j + 1) * 128], Wo[:],
                                     start=True, stop=True)
                osb = finp.tile([128, MAX_GROUP_BINS * 128], f16, tag="osb")
                nc.scalar.copy(osb[:, 0:ng * 128], fps[:, 0:ng * 128])
                b0 = g[0]
                nc.sync.dma_start(
                    t_out[:, b0 * 128:(b0 + ng) * 128], osb[:, 0:ng * 128])

            q_emitted = 0

            def need_qproj_upto(bin_hi):
                nonlocal q_emitted
                while q_emitted < min(bin_hi, nbins):
                    q_slab(q_emitted)
                    q_emitted += QSLAB

            need_qproj_upto(groups[0][-1] + 1)
            for gi in range(len(groups) + 1):
                if gi + 1 < len(groups):
                    need_qproj_upto(groups[gi + 1][-1] + 1)
                if gi < len(groups):
                    stage_a(gi)
                if gi >= 1:
                    stage_b(gi - 1)

    nc.compile()
    return nc


def _wrap16(idx, n_slots):
    w = np.zeros((16, n_slots // 16), dtype=np.int16)
    w[:, :] = idx.astype(np.int16).reshape(n_slots // 16, 16).T
    return np.tile(w, (8, 1))


def kernel(**inputs):
    query = np.asarray(inputs["query"], np.float32)
    key_in = np.asarray(inputs["key_in"], np.float32)
    value_in = np.asarray(inputs["value_in"], np.float32)
    src = np.asarray(inputs["src"]).astype(np.int64)
    dst = np.asarray(inputs["dst"]).astype(np.int64)
    ea = np.asarray(inputs["edge_attr"], np.float32).reshape(-1)
    Wq = np.asarray(inputs["Wq"], np.float32)
    Wk = np.asarray(inputs["Wk"], np.float32)
    Wv = np.asarray(inputs["Wv"], np.float32)
    Wo = np.asarray(inputs["Wo"], np.float32)
    bq = np.asarray(inputs["bq"], np.float32)
    bk = np.asarray(inputs["bk"], np.float32)
    bv = np.asarray(inputs["bv"], np.float32)
    bo = np.asarray(inputs["bo"], np.float32)
    rbf_gamma = np.asarray(inputs["rbf_gamma"], np.float32)

    nq = query.shape[0]
    nkv = key_in.shape[0]
    E = src.shape[0]
    nkv_pad = ((nkv + 511) // 512) * 512

    gamma = np.maximum(rbf_gamma, np.float32(1e-8))
    rbf_all = (np.exp(-(gamma[None, :]) * (ea[:, None] ** 2))
               / np.float32(SCALE)).astype(np.float32)

    order = np.argsort(src, kind="stable")
    ssrc = src[order]
    sdst = dst[order]
    srbf = rbf_all[order]

    deg = np.bincount(src, minlength=nq).astype(np.int64)
    e_starts = np.zeros(nq + 1, dtype=np.int64)
    np.cumsum(deg, out=e_starts[1:])

    # degree-sorted rows (deg>0), dealt round-robin to cores
    rows_nz = np.nonzero(deg)[0]
    order_rows = rows_nz[np.argsort(deg[rows_nz], kind="stable")]
    core_rows = [order_rows[c::NCORES] for c in range(NCORES)]
    nrows_max = max(len(r) for r in core_rows)
    nbins = (nrows_max + 127) // 128

    # common bin template: D_b = max degree over all cores' rows in bin b
    D_list = []
    for b in range(nbins):
        mx = 1
        for c in range(NCORES):
            seg = core_rows[c][b * 128:(b + 1) * 128]
            if len(seg):
                mx = max(mx, int(deg[seg].max()))
        D_list.append(mx)
    slotbase = np.concatenate([[0], np.cumsum(D_list)]).astype(int)
    total_slots = int(slotbase[-1])
    if total_slots % 2:
        D_list[-1] += 1
        slotbase = np.concatenate([[0], np.cumsum(D_list)]).astype(int)
        total_slots = int(slotbase[-1])

    # gather groups: consecutive bins, caps on bins and slot-chunks
    groups = []
    cur = []
    cur_slots = 0
    for b in range(nbins):
        if cur and (len(cur) >= MAX_GROUP_BINS
                    or cur_slots + D_list[b] > MAX_GROUP_SLOTS):
            groups.append(tuple(cur))
            cur = []
            cur_slots = 0
        cur.append(b)
        cur_slots += D_list[b]
    if cur:
        groups.append(tuple(cur))

    key = (nkv_pad, nbins, tuple(D_list), tuple(groups), total_slots)
    if key not in _PROG_CACHE:
        _PROG_CACHE[key] = _build_program(nkv_pad, nbins, D_list, groups,
                                          total_slots)
    nc = _PROG_CACHE[key]

    kT_pad = np.zeros((128, nkv_pad), np.float16)
    kT_pad[:, :nkv] = key_in.T.astype(np.float16)
    vT_pad = np.zeros((128, nkv_pad), np.float16)
    vT_pad[:, :nkv] = value_in.T.astype(np.float16)
    ident_t = np.eye(128, dtype=np.float16)
    ones_t = np.ones((1, 128), np.float16)

    in_maps = []
    unpack = []
    for c in range(NCORES):
        rows_c = core_rows[c]
        qT = np.zeros((128, nbins * 128), np.float16)
        didx_cols = np.zeros((total_slots, 128), np.int64)
        rbf_cols = np.zeros((total_slots, 128, HEADS), np.float16)
        pcnt_t = np.zeros((128, nbins), np.float32)
        rows_glob = np.zeros(nbins * 128, np.int64) - 1

        for b in range(nbins):
            D = D_list[b]
            rows = rows_c[b * 128:(b + 1) * 128]
            nr = len(rows)
            # dummy rows: subtract D-1 pads so den=1, probs finite, V=0
            pcnt_t[:, b] = np.float32(D - 1)
            didx_cols[slotbase[b]:slotbase[b] + D, :] = nkv_pad
            if nr == 0:
                continue
            sb = slotbase[b]
            qT[:, b * 128:b * 128 + nr] = query[rows].T.astype(np.float16)
            rows_glob[b * 128:b * 128 + nr] = rows
            degs = deg[rows]
            pcnt_t[:nr, b] = (D - degs).astype(np.float32)
            e0 = e_starts[rows]
            sgrid = np.arange(D)[None, :]
            idx2d = e0[:, None] + sgrid
            valid = sgrid < degs[:, None]
            idx2d = np.where(valid, idx2d, 0)
            d2d = np.where(valid, sdst[idx2d], nkv_pad)
            didx_cols[sb:sb + D, :nr] = d2d.T
            r2d = np.where(valid[:, :, None], srbf[idx2d], 0.0)
            rbf_cols[sb:sb + D, :nr] = r2d.transpose(1, 0, 2)

        didx_flat = didx_cols.reshape(-1)
        in_maps.append({
            "qT": qT, "kT": kT_pad, "vT": vT_pad,
            "Wq": Wq.astype(np.float16), "Wk": Wk.astype(np.float16),
            "Wv": Wv.astype(np.float16), "Wo": Wo.astype(np.float16),
            "bq": bq.reshape(1, 128).astype(np.float16),
            "bk": bk.reshape(1, 128).astype(np.float16),
            "bv": bv.reshape(1, 128).astype(np.float16),
            "ones1": ones_t, "ident": ident_t,
            "bkv4": np.tile(np.concatenate([bk, bv]).astype(np.float16),
                            2).reshape(1, 512),
            "bq4": np.tile(bq.astype(np.float16), 4).reshape(1, 512),
            "rbfm": np.ascontiguousarray(
                rbf_cols.transpose(1, 0, 2)).reshape(128, -1),
            "pcnt": pcnt_t,
            "didx": _wrap16(didx_flat, total_slots * 128),
        })
        unpack.append(rows_glob)

    from concourse.bass_utils import run_bass_kernel_spmd
    g = globals()
    g["LAST_NC"] = nc
    g["LAST_INMAPS"] = in_maps
    res = run_bass_kernel_spmd(nc, in_maps, list(range(NCORES)),
                               trace=g.get("TRACE", False))
    g["LAST_RESULTS"] = res

    out = np.zeros((nq, HIDDEN), np.float32)
    for c in range(NCORES):
        o = np.asarray(res.results[c]["out"]).astype(np.float32)
        o = o.reshape(128, -1, 128).transpose(1, 0, 2).reshape(-1, 128)
        valid = unpack[c] >= 0
        out[unpack[c][valid]] = o[valid]
    out += bo[None, :]
    return out
